# revision 3
# baseline (speedup 1.0000x reference)
"""Trainium2 Bass kernel for nn_DerivedMLP (1,2,64,2,512,512) -> (1,64).

Computation (per the original nn.Module):
  x: (1, 2, 64, 2, 512, 512) f32; channel 0 of dim1 holds the [n, phi] fields.
  gamma[t] = -mean(n[t] * d(phi[t])/dy)        (numpy.gradient semantics on y)
  out      = w2 @ gelu_tanh(w1 @ [derived; gamma] + b1) + b2   (1x1 conv over t)

Sharding: the computation is fully independent per time step t, so t is
sharded across the 8 NeuronCores: core k handles t in [8k, 8k+8).  Zero
communication; each core reads a contiguous 16 MB slice of x (only channel 0
is ever read).  The tiny MLP weights are replicated (pre-transposed /
broadcast host-side) and the host concatenates the per-core (1, 8) outputs.

Per-core kernel. The problem is memory-bound: 16 MB of HBM reads at the
~320-350 GB/s this hardware sustains is ~48-52 us, so the design keeps the
DMA stream saturated and minimizes the serial head/tail around it:
  - HW-calibrated stream (For_i microbenches): few BIG DMAs on ONE HWDGE
    queue (big beats chunked by ~150+ ns per extra DMA; a second queue or
    gpsimd SWDGE queue measured slower).  The host packs each of t0..t5 as
    one contiguous 2 MB block (per partition row: [n row | phi row]) so one
    DMA loads both fields of a t.
  - p6+p7 load FIRST (one 2 MB DMA) so the t6/t7 stencils run mid-stream;
    n6 and n7 stream LAST, chunked, so the per-chunk fused multiply+reduce
    tracks the arrivals and the tail only waits on the final small chunk.
  - DVE per t: central-difference stencil (one 2046-wide sub + strided
    one-sided segment edges), then ONE fused pass via scalar_tensor_tensor:
    accum_out gives the per-partition row sum of n*d directly, and the out
    operand is a stride-0 broadcast dummy so the product tensor is never
    materialized in SBUF (reduces SBUF-write/DMA contention).
  - MLP in transposed (t, hidden) layout: hpT (8,4) PSUM accumulates
    derb^T@[b1;w1_derived] (early) + acc^T@w1_gamma_bcast (tail: one matmul
    turns the (128,8) partial-sum matrix straight into hidden
    pre-activations); ACT Gelu_apprx_tanh (exact tanh-gelu match, table
    pre-warmed); one DVE scalar_tensor_tensor with [w2|b2] finishes
    out^T (8,1), which DMAs directly to the (1,8) output row.
  - gamma's -0.5/dx/N scale is folded into the broadcast w1 column
    host-side; b1/b2 are folded via ones-row/ones-column tricks.
"""

import os
import sys

import numpy as np

for _p in ("/opt/trn_rl_repo",):
    if os.path.isdir(_p) and _p not in sys.path:
        sys.path.insert(0, _p)

# Defensive: the bass execution path runs through the axon PJRT plugin; if the
# caller's env pinned JAX_PLATFORMS without axon (and jax isn't initialized
# yet), restore it so jax.devices() can see the NeuronCores.
if (
    os.environ.get("AXON_H4_ENABLED") == "1"
    or os.environ.get("AXON_TERMINAL_JOB_NAME")
) and "jax" not in sys.modules:
    _plat = os.environ.get("JAX_PLATFORMS", "")
    if _plat and "axon" not in _plat:
        os.environ["JAX_PLATFORMS"] = "axon," + _plat

# ---- problem constants (hardcoded per contract) ----
DX = 0.1
B, C, T, V, NX, NY = 1, 2, 64, 2, 512, 512
N_CORES = 8
T_PER_CORE = T // N_CORES  # 8
P = 128                    # SBUF partitions
FREE = (NX * NY) // P      # 2048 f32 per partition = whole 512x512 image
SEG = NY                   # 512; partition rows hold 4 y-segments each
GAMMA_SCALE = -(0.5 / DX) / float(NX * NY)

N_MERGED = 6               # t0..t5 as merged 2MB blocks
# column layout of the flat per-core DRAM tensor (128, 32768):
#   [t*4096 + 0:2048]    n_t   for t in 0..5
#   [t*4096 + 2048:4096] phi_t for t in 0..5
#   [24576:26624] p6   [26624:28672] p7   [28672:30720] n6   [30720:32768] n7
COLS = N_MERGED * 2 * FREE + 4 * FREE  # 32768
OFF_P6 = N_MERGED * 2 * FREE
OFF_P7 = OFF_P6 + FREE
OFF_N6 = OFF_P7 + FREE
OFF_N7 = OFF_N6 + FREE

# chunk plans for the two late n fields (tail latency vs per-DMA overhead)
N6_PLAN = [1024, 1024]
N7_PLAN = [1024, 512, 256, 256]

_CACHE = {}


def _build_nc(repeat=None):
    import concourse.mybir as mybir
    import concourse.tile as tile
    import concourse.bass as bass
    from concourse import bacc

    f32 = mybir.dt.float32
    sub = mybir.AluOpType.subtract
    mult = mybir.AluOpType.mult
    Gelu = mybir.ActivationFunctionType.Gelu_apprx_tanh

    nc = bacc.Bacc(
        "TRN2", target_bir_lowering=False, debug=False, num_devices=N_CORES
    )

    xs = nc.dram_tensor("xs", (P, COLS), f32, kind="ExternalInput").ap()
    der = nc.dram_tensor("derived", (1, T_PER_CORE), f32, kind="ExternalInput").ap()
    w1b = nc.dram_tensor("w1b", (2, 4), f32, kind="ExternalInput").ap()
    w1c0b = nc.dram_tensor("w1c0b", (P, 4), f32, kind="ExternalInput").ap()
    w2bT = nc.dram_tensor("w2bT", (T_PER_CORE, 5), f32, kind="ExternalInput").ap()
    b2b = nc.dram_tensor("b2b", (T_PER_CORE, 1), f32, kind="ExternalInput").ap()
    out = nc.dram_tensor("out", (1, T_PER_CORE), f32, kind="ExternalOutput").ap()

    with tile.TileContext(nc) as tc:
        with (
            tc.tile_pool(name="big", bufs=1) as big,
            tc.tile_pool(name="small", bufs=1) as small,
            tc.tile_pool(name="ps", bufs=1, space=bass.MemorySpace.PSUM) as ps,
        ):
            # ---- persistent tiles ----
            w1b_s = small.tile([2, 4], f32)
            w1c0b_s = small.tile([P, 4], f32)
            w2bT_s = small.tile([T_PER_CORE, 5], f32)
            derb = small.tile([2, T_PER_CORE], f32)  # row0 ones, row1 derived
            acc = small.tile([P, T_PER_CORE], f32)
            chain = small.tile([P, 16], f32)
            hgT = small.tile([T_PER_CORE, 5], f32)  # cols 0-3 gelu, col 4 = b2
            resT = small.tile([T_PER_CORE, 1], f32)
            sc5 = small.tile([T_PER_CORE, 5], f32)
            warm = small.tile([1, 1], f32)

            dA = big.tile([P, FREE], f32, name="dA")
            dB = big.tile([P, FREE], f32, name="dB")
            d6 = big.tile([P, FREE], f32, name="d6")
            d7 = big.tile([P, FREE], f32, name="d7")
            bdum = big.tile([P, 1], f32, name="bdum")

            m = [big.tile([P, 2 * FREE], f32, name=f"m{t}") for t in range(N_MERGED)]
            p67t = big.tile([P, 2 * FREE], f32, name="p67t")
            n6t = [big.tile([P, w], f32, name=f"n6_{c}") for c, w in enumerate(N6_PLAN)]
            n7t = [big.tile([P, w], f32, name=f"n7_{c}") for c, w in enumerate(N7_PLAN)]

            # ---- small loads on the ACT queue (SP owns the big stream) ----
            nc.scalar.dma_start(w1b_s[:], w1b[:])
            nc.scalar.dma_start(w1c0b_s[:], w1c0b[:])
            nc.scalar.dma_start(w2bT_s[:], w2bT[:])
            nc.scalar.dma_start(hgT[:, 4:5], b2b[:])
            # ones row must be partition 0 (engine APs start at partition 0);
            # derived lands in partition 1 via DMA
            nc.scalar.dma_start(derb[1:2, :], der[:])

            nc.vector.memset(derb[0:1, :], 1.0)
            nc.vector.memset(warm[:], 0.0)
            # hoist the ACT Gelu function-table load off the kernel tail
            nc.scalar.activation(warm[:], warm[:], Gelu, bias=0.0, scale=1.0)

            import contextlib
            loop_cm = tc.For_i(0, repeat) if repeat else contextlib.nullcontext()
            with loop_cm:
                # ---- big DMA stream, all on the SP HWDGE queue ----
                def dma(dst, off, width):
                    nc.sync.dma_start(dst, xs[:, off : off + width])

                dma(p67t[:], OFF_P6, 2 * FREE)     # p6+p7: one 2MB DMA, first
                for t in range(N_MERGED):
                    dma(m[t][:], t * 2 * FREE, 2 * FREE)
                off = OFF_N6
                for c, w in enumerate(N6_PLAN):
                    dma(n6t[c][:], off, w)
                    off += w
                off = OFF_N7
                for c, w in enumerate(N7_PLAN):
                    dma(n7t[c][:], off, w)
                    off += w

                # ---- DVE program ----
                def stencil(ptile, d):
                    # central diff with one-sided doubled edges; segment
                    # edge columns fixed with strided ops
                    nc.vector.tensor_tensor(
                        d[:, 1 : FREE - 1], ptile[:, 2:FREE], ptile[:, 0 : FREE - 2],
                        sub,
                    )
                    nc.vector.tensor_tensor(
                        d[:, 0:FREE:SEG], ptile[:, 1:FREE:SEG], ptile[:, 0:FREE:SEG],
                        sub,
                    )
                    nc.vector.tensor_scalar_mul(d[:, 0:FREE:SEG], d[:, 0:FREE:SEG], 2.0)
                    nc.vector.tensor_tensor(
                        d[:, SEG - 1 : FREE : SEG],
                        ptile[:, SEG - 1 : FREE : SEG],
                        ptile[:, SEG - 2 : FREE : SEG],
                        sub,
                    )
                    nc.vector.tensor_scalar_mul(
                        d[:, SEG - 1 : FREE : SEG], d[:, SEG - 1 : FREE : SEG], 2.0
                    )

                def fused_full(ntile, d, t):
                    # acc[:,t] = rowsum(n*d) in one DVE pass; the product
                    # tensor itself collapses into a broadcast dummy column
                    nc.vector.scalar_tensor_tensor(
                        bdum.broadcast_to(d[:].shape), ntile, 1.0, d[:],
                        mult, mult, accum_out=acc[:, t : t + 1],
                    )

                def fused_chunked(ntiles_, plan, d, t):
                    off = 0
                    for c, w in enumerate(plan):
                        nc.vector.scalar_tensor_tensor(
                            bdum.broadcast_to(ntiles_[c][:].shape),
                            ntiles_[c][:], 1.0, d[:, off : off + w],
                            mult, mult, accum_out=chain[:, c : c + 1],
                        )
                        off += w
                    nc.vector.reduce_sum(
                        acc[:, t : t + 1], chain[:, 0 : len(plan)],
                        axis=mybir.AxisListType.X,
                    )

                # t6/t7 stencils first (their p data arrives first)
                stencil(p67t[:, 0:FREE], d6)
                stencil(p67t[:, FREE : 2 * FREE], d7)
                for t in range(N_MERGED):
                    d = dA if t % 2 == 0 else dB
                    stencil(m[t][:, FREE : 2 * FREE], d)
                    fused_full(m[t][:, 0:FREE], d, t)
                fused_chunked(n6t, N6_PLAN, d6, 6)
                fused_chunked(n7t, N7_PLAN, d7, 7)

                # ---- MLP (transposed: t on partitions) ----
                hpT = ps.tile([T_PER_CORE, 4], f32)
                # hpT[t,j] = derived[t]*w1[j,0] + b1[j]   (early)
                nc.tensor.matmul(hpT[:], derb[:], w1b_s[:], start=True, stop=False)
                # hpT[t,j] += sum_p acc[p,t] * GAMMA_SCALE*w1[j,1]   (tail)
                nc.tensor.matmul(hpT[:], acc[:], w1c0b_s[:], start=False, stop=True)
                nc.scalar.activation(hgT[:, 0:4], hpT[:], Gelu, bias=0.0, scale=1.0)
                # resT[t] = sum_j hgT[t,j]*w2bT[t,j]   (col 4: b2 * 1)
                nc.vector.scalar_tensor_tensor(
                    sc5[:], hgT[:], 1.0, w2bT_s[:], mult, mult, accum_out=resT[:]
                )
                nc.sync.dma_start(out[:], resT[:])

    nc.compile()
    return nc


def get_nc():
    if "nc" not in _CACHE:
        _CACHE["nc"] = _build_nc()
    return _CACHE["nc"]


def make_in_maps(x, input_derived, w1, b1, w2, b2):
    x = np.asarray(x, dtype=np.float32)
    input_derived = np.asarray(input_derived, dtype=np.float32)
    w1 = np.asarray(w1, np.float32)
    # reference: h_j = w1[j,0]*derived + w1[j,1]*gamma
    w1b = np.ascontiguousarray(
        np.stack([np.asarray(b1, np.float32).reshape(4), w1[:, 0]])
    )  # (2,4): row0 b1 (pairs with the ones row), row1 derived weights
    w1c0b = np.ascontiguousarray(
        np.tile(w1[:, 1].reshape(1, 4) * np.float32(GAMMA_SCALE), (P, 1))
    )
    w2row = np.concatenate(
        [np.asarray(w2, np.float32).reshape(4), np.ones(1, np.float32)]
    )
    w2bT = np.ascontiguousarray(np.tile(w2row.reshape(1, 5), (T_PER_CORE, 1)))
    b2b = np.ascontiguousarray(
        np.full((T_PER_CORE, 1), np.float32(np.asarray(b2).reshape(())), np.float32)
    )

    x0 = x[0, 0]  # (64, 2, 512, 512): [t, v, nx, ny]
    in_maps = []
    for k in range(N_CORES):
        t0 = k * T_PER_CORE
        blk = x0[t0 : t0 + T_PER_CORE].reshape(T_PER_CORE, 2, P, FREE)
        xs_k = np.empty((P, COLS), np.float32)
        for t in range(N_MERGED):
            xs_k[:, t * 2 * FREE : t * 2 * FREE + FREE] = blk[t, 0]
            xs_k[:, t * 2 * FREE + FREE : (t + 1) * 2 * FREE] = blk[t, 1]
        xs_k[:, OFF_P6 : OFF_P6 + FREE] = blk[6, 1]
        xs_k[:, OFF_P7 : OFF_P7 + FREE] = blk[7, 1]
        xs_k[:, OFF_N6 : OFF_N6 + FREE] = blk[6, 0]
        xs_k[:, OFF_N7 : OFF_N7 + FREE] = blk[7, 0]
        der_k = np.ascontiguousarray(input_derived[:, t0 : t0 + T_PER_CORE])
        in_maps.append(
            {
                "xs": xs_k,
                "derived": der_k,
                "w1b": w1b,
                "w1c0b": w1c0b,
                "w2bT": w2bT,
                "b2b": b2b,
            }
        )
    return in_maps


def kernel(x, input_derived, w1, b1, w2, b2, trace=False):
    from concourse.bass_utils import run_bass_kernel_spmd

    nc = get_nc()
    in_maps = make_in_maps(x, input_derived, w1, b1, w2, b2)
    results = run_bass_kernel_spmd(
        nc, in_maps, core_ids=list(range(N_CORES)), trace=trace
    )
    _CACHE["last_results"] = results
    return np.concatenate([r["out"] for r in results.results], axis=1)
